# revision 9
# baseline (speedup 1.0000x reference)
"""Trainium2 Bass kernel for ColumnParallelLinearWithTopping.

Computes  y[t] = x[t] @ (W_base.T + DeltaW[j] + A[j] @ B[j]),  j = weight_indices[t]

Strategy (8-core tensor parallel over the output dim, 512 cols/core):
  * Host: stable-argsort tokens by adapter id, pack x rows grouped by
    adapter (each group padded to a multiple of 128 rows), and ship x
    TRANSPOSED ([D_IN, T_pad]) so the device never transposes activations.
    W_base is pre-transposed to [D_IN, D_OUT]; A to [RANK, D_IN]
    (layout-only transforms).
  * Device (per core, SPMD): for each adapter a, build the effective
    weight  W_full[a] = W_base.T + DeltaW[a] + A[a] @ B[a]  (column shard)
    in SBUF (PE matmul for A@B + DVE adds), then a grouped GEMM over that
    adapter's tokens, 4 blocks (512 tokens) at a time:
        psum_y[b][tok,512] += xT[k, tokens_b].T @ W_full[a][k]
    All matmuls run in float32r (full-rate fp32 PE mode).
  * Host: concatenate per-core column shards and undo the permutation.
"""
import os
from contextlib import ExitStack

import numpy as np

import concourse.bass as bass
import concourse.mybir as mybir
import concourse.tile as tile
from concourse import bacc
from concourse.bass_utils import run_bass_kernel_spmd

T, D_IN, D_OUT = 8192, 4096, 4096
N_ADAPT, RANK = 8, 16
N_CORES = 8
P = 128
SHARD = D_OUT // N_CORES          # 512 output cols per core
KT = D_IN // P                    # 32 contraction tiles
GRP = 8                           # token blocks per GEMM group
F32 = mybir.dt.float32
F32R = mybir.dt.float32r

USE_F32R = os.environ.get("KERNEL_FP32R", "1") == "1"
DT = F32R if USE_F32R else F32   # dtype of the x / weight data path

_build_cache: dict = {}


def _build(nb: tuple):
    """Build + compile the SPMD program for per-adapter block counts nb."""
    t_pad = sum(nb) * P
    nc = bacc.Bacc("TRN2", target_bir_lowering=False, debug=False)
    xt = nc.dram_tensor("xt", [D_IN, t_pad], DT, kind="ExternalInput").ap()
    wbt = nc.dram_tensor("wbt", [P, KT * SHARD], DT, kind="ExternalInput").ap()
    dw = nc.dram_tensor("dw", [N_ADAPT, KT // 4, P, 4 * SHARD], DT, kind="ExternalInput").ap()
    at = nc.dram_tensor("at", [N_ADAPT, RANK, D_IN], DT, kind="ExternalInput").ap()
    bb = nc.dram_tensor("bb", [N_ADAPT, RANK, SHARD], DT, kind="ExternalInput").ap()
    y = nc.dram_tensor("y", [t_pad, SHARD], F32, kind="ExternalOutput").ap()

    with tile.TileContext(nc) as tc, ExitStack() as ctx:
        const = ctx.enter_context(tc.tile_pool(name="const", bufs=1))
        wf_pool = ctx.enter_context(tc.tile_pool(name="wf", bufs=34))
        xt_pool = ctx.enter_context(tc.tile_pool(name="xtp", bufs=4))
        dwt_pool = ctx.enter_context(tc.tile_pool(name="dwt", bufs=2))
        ab_pool = ctx.enter_context(tc.tile_pool(name="ab", bufs=1))
        y_pool = ctx.enter_context(tc.tile_pool(name="yo", bufs=4))
        psum_y = ctx.enter_context(tc.tile_pool(name="psum_y", bufs=1, space="PSUM"))

        wbt_sb = const.tile([P, KT, SHARD], DT, name="wbt_sb")
        nc.scalar.dma_start(wbt_sb, wbt.rearrange("p (kt n) -> p kt n", kt=KT))

        blk_base = 0
        for a in range(N_ADAPT):
            if nb[a] == 0:
                continue
            at_sb = ab_pool.tile([RANK, D_IN], DT, name="at_sb")
            nc.scalar.dma_start(at_sb, at[a])
            b_sb = ab_pool.tile([RANK, SHARD], DT, name="b_sb")
            nc.scalar.dma_start(b_sb, bb[a])

            # ---- build W_full[a] in SBUF: 32 tiles of [128, SHARD] ----
            wtiles = []
            for k4 in range(KT // 4):
                dwt = dwt_pool.tile([P, 4, SHARD], DT, name="dwt")
                nc.scalar.dma_start(
                    dwt, dw[a, k4].rearrange("p (i n) -> p i n", i=4))
                for i in range(4):
                    k = k4 * 4 + i
                    ab_ps = psum_y.tile([P, SHARD], F32, name="ab_ps",
                                        tag=f"py{k % GRP}", bufs=1)
                    nc.tensor.matmul(
                        ab_ps,
                        at_sb[:, k * P:(k + 1) * P],
                        b_sb,
                        start=True, stop=True,
                    )
                    wf = wf_pool.tile([P, SHARD], DT, name="wf")
                    nc.vector.tensor_add(wf, ab_ps, dwt[:, i, :])
                    nc.vector.tensor_add(wf, wf, wbt_sb[:, k, :])
                    wtiles.append(wf)

            # ---- grouped GEMM: 4 token blocks (512 tokens) at a time ----
            blk = 0
            while blk < nb[a]:
                g = min(GRP, nb[a] - blk)
                tok0 = (blk_base + blk) * P
                W = g * P
                psums = [psum_y.tile([P, SHARD], F32, name=f"py{b}",
                                     tag=f"py{b}", bufs=1)
                         for b in range(g)]
                for k in range(KT):
                    xt_sb = xt_pool.tile([P, GRP * P], DT, name="xt_sb")
                    nc.sync.dma_start(
                        xt_sb[:, :W], xt[k * P:(k + 1) * P, tok0:tok0 + W])
                    for b in range(g):
                        nc.tensor.matmul(
                            psums[b],
                            xt_sb[:, b * P:(b + 1) * P],
                            wtiles[k],
                            start=(k == 0), stop=(k == KT - 1),
                        )
                for b in range(g):
                    y_sb = y_pool.tile([P, SHARD], F32, name="y_sb")
                    nc.vector.tensor_copy(y_sb, psums[b])
                    nc.scalar.dma_start(
                        y[tok0 + b * P:tok0 + (b + 1) * P, :], y_sb)
                blk += g
            blk_base += nb[a]

    nc.compile()
    return nc, t_pad


def kernel(x, weight_indices, W_base, A_buffer, B_buffer, DeltaW):
    x = np.asarray(x, dtype=np.float32)
    idx = np.asarray(weight_indices).astype(np.int64)
    W_base = np.asarray(W_base, dtype=np.float32)
    A_buffer = np.asarray(A_buffer, dtype=np.float32)
    B_buffer = np.asarray(B_buffer, dtype=np.float32)
    DeltaW = np.asarray(DeltaW, dtype=np.float32)

    order = np.argsort(idx, kind="stable")
    counts = np.bincount(idx, minlength=N_ADAPT)
    nb = tuple(int(-(-c // P)) for c in counts)
    t_pad = sum(nb) * P

    key = nb
    if key not in _build_cache:
        _build_cache[key] = _build(nb)
    nc, _ = _build_cache[key]

    # pack x columns (transposed) grouped by adapter, pad to 128-row blocks
    xT = np.ascontiguousarray(x.T)                  # [D_IN, T]
    xt_packed = np.zeros((D_IN, t_pad), dtype=np.float32)
    seg_dst = []          # (dst_row0, count, sorted_token_slice_start)
    cum = np.concatenate([[0], np.cumsum(counts)])
    row0 = 0
    for a in range(N_ADAPT):
        c = int(counts[a])
        if c:
            xt_packed[:, row0:row0 + c] = xT[:, order[cum[a]:cum[a] + c]]
        seg_dst.append((row0, c, int(cum[a])))
        row0 += nb[a] * P

    wbT = np.ascontiguousarray(W_base.T)                     # [D_IN, D_OUT]
    # [D_OUT/SHARD][P, KT*SHARD]: partition-major so DMA rows are contiguous
    wb_r = wbT.reshape(KT, P, D_OUT).transpose(1, 0, 2)      # [P, KT, D_OUT]
    atT = np.ascontiguousarray(A_buffer.transpose(0, 2, 1))  # [A, RANK, D_IN]

    in_maps = []
    for c in range(N_CORES):
        sl = slice(c * SHARD, (c + 1) * SHARD)
        in_maps.append({
            "xt": xt_packed,
            "wbt": np.ascontiguousarray(
                wb_r[:, :, sl]).reshape(P, KT * SHARD),
            "dw": np.ascontiguousarray(
                DeltaW[:, :, sl].reshape(N_ADAPT, KT // 4, 4, P, SHARD)
                .transpose(0, 1, 3, 2, 4)).reshape(
                    N_ADAPT, KT // 4, P, 4 * SHARD),
            "at": atT,
            "bb": np.ascontiguousarray(B_buffer[:, :, sl]),
        })

    global _last_in_maps
    _last_in_maps = in_maps
    res = run_bass_kernel_spmd(nc, in_maps, core_ids=list(range(N_CORES)))
    y_packed = np.concatenate(
        [res.results[c]["y"] for c in range(N_CORES)], axis=1)  # [t_pad, D_OUT]

    out = np.empty((T, D_OUT), dtype=np.float32)
    for a in range(N_ADAPT):
        row0, c, s = seg_dst[a]
        if c:
            out[order[s:s + c]] = y_packed[row0:row0 + c]
    return out


# revision 10
# speedup vs baseline: 1.2794x; 1.2794x over previous
"""Trainium2 Bass kernel for ColumnParallelLinearWithTopping.

Computes  y[t] = x[t] @ (W_base.T + DeltaW[j] + A[j] @ B[j]),  j = weight_indices[t]

Strategy (8-core tensor parallel over the output dim, 512 cols/core):
  * Host: stable-argsort tokens by adapter id, pack x rows grouped by
    adapter (each group padded to a multiple of 128 rows), and ship x
    TRANSPOSED ([D_IN, T_pad]) so the device never transposes activations.
    W_base is pre-transposed to [D_IN, D_OUT]; A to [RANK, D_IN]
    (layout-only transforms).
  * Device (per core, SPMD): for each adapter a, build the effective
    weight  W_full[a] = W_base.T + DeltaW[a] + A[a] @ B[a]  (column shard)
    in SBUF (PE matmul for A@B + DVE adds), then a grouped GEMM over that
    adapter's tokens, 4 blocks (512 tokens) at a time:
        psum_y[b][tok,512] += xT[k, tokens_b].T @ W_full[a][k]
    All matmuls run in float32r (full-rate fp32 PE mode).
  * Host: concatenate per-core column shards and undo the permutation.
"""
import os
from contextlib import ExitStack

import numpy as np

import concourse.bass as bass
import concourse.mybir as mybir
import concourse.tile as tile
from concourse import bacc
from concourse.bass_utils import run_bass_kernel_spmd

T, D_IN, D_OUT = 8192, 4096, 4096
N_ADAPT, RANK = 8, 16
N_CORES = 8
P = 128
SHARD = D_OUT // N_CORES          # 512 output cols per core
KT = D_IN // P                    # 32 contraction tiles
GRP = 6                           # token blocks per GEMM group
F32 = mybir.dt.float32
F32R = mybir.dt.float32r

USE_F32R = os.environ.get("KERNEL_FP32R", "1") == "1"
DT = F32R if USE_F32R else F32   # dtype of the x / weight data path

_build_cache: dict = {}


def _build(nb: tuple):
    """Build + compile the SPMD program for per-adapter block counts nb."""
    t_pad = sum(nb) * P
    nc = bacc.Bacc("TRN2", target_bir_lowering=False, debug=False)
    xt = nc.dram_tensor("xt", [D_IN, t_pad], DT, kind="ExternalInput").ap()
    wbt = nc.dram_tensor("wbt", [P, KT * SHARD], DT, kind="ExternalInput").ap()
    dw = nc.dram_tensor("dw", [N_ADAPT, KT // 4, P, 4 * SHARD], DT, kind="ExternalInput").ap()
    at = nc.dram_tensor("at", [N_ADAPT, RANK, D_IN], DT, kind="ExternalInput").ap()
    bb = nc.dram_tensor("bb", [N_ADAPT, RANK, SHARD], DT, kind="ExternalInput").ap()
    y = nc.dram_tensor("y", [t_pad, SHARD], F32, kind="ExternalOutput").ap()

    with tile.TileContext(nc) as tc, ExitStack() as ctx:
        const = ctx.enter_context(tc.tile_pool(name="const", bufs=1))
        wf_pool = ctx.enter_context(tc.tile_pool(name="wf", bufs=34))
        xt_pool = ctx.enter_context(tc.tile_pool(name="xtp", bufs=6))
        dwt_pool = ctx.enter_context(tc.tile_pool(name="dwt", bufs=2))
        ab_pool = ctx.enter_context(tc.tile_pool(name="ab", bufs=1))
        y_pool = ctx.enter_context(tc.tile_pool(name="yo", bufs=3))
        psum_y = ctx.enter_context(tc.tile_pool(name="psum_y", bufs=1, space="PSUM"))
        psum_m = ctx.enter_context(tc.tile_pool(name="psum_m", bufs=2, space="PSUM"))

        wbt_sb = const.tile([P, KT, SHARD], DT, name="wbt_sb")
        nc.scalar.dma_start(wbt_sb, wbt.rearrange("p (kt n) -> p kt n", kt=KT))

        blk_base = 0
        for a in range(N_ADAPT):
            if nb[a] == 0:
                continue
            at_sb = ab_pool.tile([RANK, D_IN], DT, name="at_sb")
            nc.scalar.dma_start(at_sb, at[a])
            b_sb = ab_pool.tile([RANK, SHARD], DT, name="b_sb")
            nc.scalar.dma_start(b_sb, bb[a])

            # ---- build W_full[a] in SBUF: 32 tiles of [128, SHARD] ----
            wtiles = []
            for k4 in range(KT // 4):
                dwt = dwt_pool.tile([P, 4, SHARD], DT, name="dwt")
                nc.scalar.dma_start(
                    dwt, dw[a, k4].rearrange("p (i n) -> p i n", i=4))
                for i in range(4):
                    k = k4 * 4 + i
                    ab_ps = psum_m.tile([P, SHARD], F32, name="ab_ps")
                    nc.tensor.matmul(
                        ab_ps,
                        at_sb[:, k * P:(k + 1) * P],
                        b_sb,
                        start=True, stop=True,
                    )
                    wf = wf_pool.tile([P, SHARD], DT, name="wf")
                    nc.vector.tensor_add(wf, ab_ps, dwt[:, i, :])
                    nc.vector.tensor_add(wf, wf, wbt_sb[:, k, :])
                    wtiles.append(wf)

            # ---- grouped GEMM: 4 token blocks (512 tokens) at a time ----
            blk = 0
            while blk < nb[a]:
                g = min(GRP, nb[a] - blk)
                tok0 = (blk_base + blk) * P
                W = g * P
                psums = [psum_y.tile([P, SHARD], F32, name=f"py{b}",
                                     tag=f"py{b}", bufs=1)
                         for b in range(g)]
                for k in range(KT):
                    xt_sb = xt_pool.tile([P, GRP * P], DT, name="xt_sb")
                    nc.sync.dma_start(
                        xt_sb[:, :W], xt[k * P:(k + 1) * P, tok0:tok0 + W])
                    for b in range(g):
                        nc.tensor.matmul(
                            psums[b],
                            xt_sb[:, b * P:(b + 1) * P],
                            wtiles[k],
                            start=(k == 0), stop=(k == KT - 1),
                        )
                for b in range(g):
                    y_sb = y_pool.tile([P, SHARD], F32, name="y_sb")
                    nc.vector.tensor_copy(y_sb, psums[b])
                    nc.scalar.dma_start(
                        y[tok0 + b * P:tok0 + (b + 1) * P, :], y_sb)
                blk += g
            blk_base += nb[a]

    nc.compile()
    return nc, t_pad


def kernel(x, weight_indices, W_base, A_buffer, B_buffer, DeltaW):
    x = np.asarray(x, dtype=np.float32)
    idx = np.asarray(weight_indices).astype(np.int64)
    W_base = np.asarray(W_base, dtype=np.float32)
    A_buffer = np.asarray(A_buffer, dtype=np.float32)
    B_buffer = np.asarray(B_buffer, dtype=np.float32)
    DeltaW = np.asarray(DeltaW, dtype=np.float32)

    order = np.argsort(idx, kind="stable")
    counts = np.bincount(idx, minlength=N_ADAPT)
    nb = tuple(int(-(-c // P)) for c in counts)
    t_pad = sum(nb) * P

    key = nb
    if key not in _build_cache:
        _build_cache[key] = _build(nb)
    nc, _ = _build_cache[key]

    # pack x columns (transposed) grouped by adapter, pad to 128-row blocks
    xT = np.ascontiguousarray(x.T)                  # [D_IN, T]
    xt_packed = np.zeros((D_IN, t_pad), dtype=np.float32)
    seg_dst = []          # (dst_row0, count, sorted_token_slice_start)
    cum = np.concatenate([[0], np.cumsum(counts)])
    row0 = 0
    for a in range(N_ADAPT):
        c = int(counts[a])
        if c:
            xt_packed[:, row0:row0 + c] = xT[:, order[cum[a]:cum[a] + c]]
        seg_dst.append((row0, c, int(cum[a])))
        row0 += nb[a] * P

    wbT = np.ascontiguousarray(W_base.T)                     # [D_IN, D_OUT]
    # [D_OUT/SHARD][P, KT*SHARD]: partition-major so DMA rows are contiguous
    wb_r = wbT.reshape(KT, P, D_OUT).transpose(1, 0, 2)      # [P, KT, D_OUT]
    atT = np.ascontiguousarray(A_buffer.transpose(0, 2, 1))  # [A, RANK, D_IN]

    in_maps = []
    for c in range(N_CORES):
        sl = slice(c * SHARD, (c + 1) * SHARD)
        in_maps.append({
            "xt": xt_packed,
            "wbt": np.ascontiguousarray(
                wb_r[:, :, sl]).reshape(P, KT * SHARD),
            "dw": np.ascontiguousarray(
                DeltaW[:, :, sl].reshape(N_ADAPT, KT // 4, 4, P, SHARD)
                .transpose(0, 1, 3, 2, 4)).reshape(
                    N_ADAPT, KT // 4, P, 4 * SHARD),
            "at": atT,
            "bb": np.ascontiguousarray(B_buffer[:, :, sl]),
        })

    global _last_in_maps
    _last_in_maps = in_maps
    res = run_bass_kernel_spmd(nc, in_maps, core_ids=list(range(N_CORES)))
    y_packed = np.concatenate(
        [res.results[c]["y"] for c in range(N_CORES)], axis=1)  # [t_pad, D_OUT]

    out = np.empty((T, D_OUT), dtype=np.float32)
    for a in range(N_ADAPT):
        row0, c, s = seg_dst[a]
        if c:
            out[order[s:s + c]] = y_packed[row0:row0 + c]
    return out
